# revision 16
# baseline (speedup 1.0000x reference)
"""MoE expert-parallel kernel for 8 TRN2 NeuronCores.

Problem: out[t] = sum_e w_e[t] * gelu(x[t] @ w1[e]) @ w2[e], top-2 routing,
8 experts == 8 cores. Strategy: expert parallelism with the dispatch/combine
("all-to-all") done on host — each core runs a dense FFN for exactly one
expert over the tokens routed to it (padded to a common capacity C), with
w1/w2 resident in SBUF as bf16 and all matmuls at bf16 rate with fp32
accumulation.
"""


import sys
import types

import numpy as np
import ml_dtypes

from concourse import bacc, bass, mybir, tile
from concourse.bass_utils import run_bass_kernel_spmd


def _harden_trace_path():
    """If BASS_TRACE is set in the environment, run_bass_kernel_spmd imports
    antenv.axon_hooks, which is missing on this image; synthesize it from
    trn_agent_boot so tracing works instead of crashing. Also make the
    artifact upload degrade to a local path when no object store is
    reachable. Both are no-ops when the real modules work."""
    try:
        try:
            from antenv import axon_hooks  # noqa: F401
        except ImportError:
            import antenv
            from trn_agent_boot.trn_boot import _ntff_profile_via_ctypes
            m = types.ModuleType("antenv.axon_hooks")
            m._hook = _ntff_profile_via_ctypes("/opt/axon/libaxon_pjrt.so")
            m.get_axon_ntff_profile_hook = lambda: m._hook
            m.set_axon_ntff_profile_hook = lambda h: setattr(m, "_hook", h)
            sys.modules["antenv.axon_hooks"] = m
            antenv.axon_hooks = m
    except Exception:
        pass
    try:
        from concourse import bass_utils as _bu
        _orig_upload = _bu.upload_artifacts

        def _safe_upload(tmpdir):
            try:
                return _orig_upload(tmpdir)
            except Exception:
                return f"local:{tmpdir}"

        _bu.upload_artifacts = _safe_upload
    except Exception:
        pass


_harden_trace_path()

N_EXPERTS = 8
D_MODEL = 1024
D_FF = 4096
N_CORES = 8

BF16 = mybir.dt.bfloat16
F32 = mybir.dt.float32

# cache of compiled graphs keyed by (capacity, d_model, d_ff)
_GRAPH_CACHE = {}
LAST_RESULTS = None  # BassKernelResults of the most recent run (for test.py)


def _token_tiles(C):
    """Split capacity C (multiple of 128) into token tiles: 512s + remainder."""
    tiles = []
    off = 0
    while C - off >= 512:
        tiles.append((off, 512))
        off += 512
    if C - off > 0:
        tiles.append((off, C - off))
        off = C
    return tiles


def _build_graph(C, d_model=D_MODEL, d_ff=D_FF):
    """Build the per-core Bass graph for capacity C tokens.

    Inputs (per core): xT [d_model, C] bf16, w1 [d_model, d_ff] bf16,
    w2 [d_ff, d_model] bf16. Output: y [C, d_model] f32.
    """
    assert d_model % 512 == 0 and d_ff % 128 == 0 and C % 128 == 0
    nc = bacc.Bacc("TRN2", target_bir_lowering=False, debug=False,
                   num_devices=N_CORES)

    KD = d_model // 128   # k-chunks for matmul1
    KF = d_ff // 128      # dff-chunks
    ND = d_model // 512   # output column chunks

    tiles = _token_tiles(C)
    NT = len(tiles)
    G1 = KF // 4          # w1 fc-groups of 4 (512 cols each)
    G2 = KF // 8          # w2 chunk-groups of 8

    # Block-contiguous input layouts: one DMA per block, with the whole block
    # contiguous per partition (full DMA throughput, ~650ns issue each), and
    # w1 delivered fc-group-major so tile-0's accumulation chains can close
    # as soon as the first ~1MB lands (subtile deps gate each matmul only on
    # the slice it reads).
    #   xT block ti: [128, KD, 512] <- x[t, k*128+p] for tile ti's tokens
    #   w1 block g:  [128, KD, 512] <- w1[k*128+p, g*512 + c]
    #   w2 block q:  [128, 8, d_model] <- w2[(8q+j)*128+p, c]
    xT_d = nc.dram_tensor("xT", [NT, 128, KD, 512], BF16,
                          kind="ExternalInput").ap()
    w1_d = nc.dram_tensor("w1", [G1, 128, KD, 512], BF16,
                          kind="ExternalInput").ap()
    w2_d = nc.dram_tensor("w2", [G2, 128, 8, d_model], BF16,
                          kind="ExternalInput").ap()
    y_d = nc.dram_tensor("y", [C, d_model], F32, kind="ExternalOutput").ap()
    gelu = mybir.ActivationFunctionType.Gelu_apprx_tanh

    with tile.TileContext(nc) as tc:
        with (
            tc.tile_pool(name="weights", bufs=1) as wpool,
            tc.tile_pool(name="xin", bufs=2) as xpool,
            tc.tile_pool(name="hbuf", bufs=1) as hpool,
            tc.tile_pool(name="yout", bufs=4) as ypool,
            tc.tile_pool(name="ps1", bufs=4, space="PSUM") as ps1pool,
            tc.tile_pool(name="ps2", bufs=4, space="PSUM") as ps2pool,
        ):
            # --- DMA order matters: x tile 0 first, then w1 (k-ascending, so
            # tile-0 matmuls can start as chunks land), then x tile 1, then w2
            # (only needed for phase B, ~60us in). All on the sync queue so
            # order is strict and HBM bandwidth isn't split. y-out DMAs go on
            # gpsimd's queue.
            x_tiles_sb = {}

            def load_x(ti, t0, TT, eng=None):
                eng = eng or nc.sync
                xt = xpool.tile([128, KD, 512], BF16, name="xsb", tag="xsb")
                eng.dma_start(out=xt[:], in_=xT_d[ti])
                x_tiles_sb[ti] = xt

            # Warmup: a short dummy matmul chain on memset scratch runs in
            # the otherwise-idle preamble/data-wait window (~6.5-10us), so
            # the PE's HAM clock-gate is already at 8/8 (2.4GHz) when the
            # first real matmul issues (cold MMs run at half clock).
            warm_sb = wpool.tile([128, 512], BF16, name="warmsb", tag="warmsb")
            nc.gpsimd.memset(warm_sb[:], 0.0)
            warm_ps = ps2pool.tile([128, 512], F32, name="warmps", tag="ps2")
            NWARM = 14
            for i in range(NWARM):
                nc.tensor.matmul(warm_ps[:], warm_sb[:, :128], warm_sb[:],
                                 start=(i == 0), stop=(i == NWARM - 1))

            # x0 first on the sync queue: strict order ahead of w1 at full
            # HBM rate (a second queue gets starved while sync streams w1).
            # Split x0 and w1-group-0 into k-halves so the first accumulation
            # chains can start after ~1.5MB instead of 2MB.
            KH = KD // 2
            x0 = xpool.tile([128, KD, 512], BF16, name="xsb", tag="xsb")
            nc.sync.dma_start(out=x0[:, :KH], in_=xT_d[0, :, :KH])
            x_tiles_sb[0] = x0

            # w1 arrives fc-group-major: each DMA delivers one 512-col group
            # across all KD k-chunks, so tile-0's accumulation chains close as
            # soon as ~1MB lands (subtile deps gate each matmul only on the
            # slice it reads).
            w1_all = wpool.tile([128, G1, KD, 512], BF16, name="w1sb",
                                tag="w1sb")
            nc.sync.dma_start(out=w1_all[:, 0, :KH], in_=w1_d[0, :, :KH])
            nc.sync.dma_start(out=x0[:, KH:], in_=xT_d[0, :, KH:])
            nc.sync.dma_start(out=w1_all[:, 0, KH:], in_=w1_d[0, :, KH:])
            for g in range(1, G1):
                nc.sync.dma_start(out=w1_all[:, g], in_=w1_d[g])

            if len(tiles) > 1:
                load_x(1, tiles[1][0], tiles[1][1])

            w2_all = wpool.tile([128, G2, 8, d_model], BF16, name="w2sb",
                                tag="w2sb")
            for q in range(G2):
                nc.sync.dma_start(out=w2_all[:, q], in_=w2_d[q])

            # hT chunk buffers (shared across token tiles, single-buffered)
            h_sb = [
                hpool.tile([128, 512], BF16, name=f"hsb{f}", tag=f"hsb{f}")
                for f in range(KF)
            ]

            for ti, (t0, TT) in enumerate(tiles):
                if ti not in x_tiles_sb:
                    load_x(ti, t0, TT)
                x_all = x_tiles_sb.pop(ti)

                # ---- matmul1 + gelu: hT[f] = gelu(w1[:,f].T @ xT) ----
                if ti == 0:
                    # k-outer over fc-groups of 4: consume w1 chunks as the
                    # DMA delivers them instead of stalling on the full w1.
                    for gi, g in enumerate(range(0, KF, 4)):
                        pool = ps1pool if gi % 2 == 0 else ps2pool
                        ptag = "ps1" if gi % 2 == 0 else "ps2"
                        pss = []
                        for f in range(g, g + 4):
                            ps1 = pool.tile([128, 512], F32, name="ps1",
                                            tag=ptag)
                            pss.append(ps1)
                        for k in range(KD):
                            for j, f in enumerate(range(g, g + 4)):
                                nc.tensor.matmul(
                                    pss[j][:, :TT],
                                    w1_all[:, f // 4, k,
                                           (f % 4) * 128:(f % 4 + 1) * 128],
                                    x_all[:, k, :TT],
                                    start=(k == 0),
                                    stop=(k == KD - 1),
                                )
                        for j, f in enumerate(range(g, g + 4)):
                            for c0 in range(0, TT, 128):
                                nc.scalar.activation(
                                    h_sb[f][:, c0:c0 + 128],
                                    pss[j][:, c0:c0 + 128], gelu)
                else:
                    for f in range(KF):
                        ps1 = ps1pool.tile([128, 512], F32, name="ps1", tag="ps1")
                        for k in range(KD):
                            nc.tensor.matmul(
                                ps1[:, :TT],
                                w1_all[:, f // 4, k,
                                       (f % 4) * 128:(f % 4 + 1) * 128],
                                x_all[:, k, :TT],
                                start=(k == 0),
                                stop=(k == KD - 1),
                            )
                        for c0 in range(0, TT, 128):
                            nc.scalar.activation(h_sb[f][:, c0:c0 + 128],
                                                 ps1[:, c0:c0 + 128], gelu)

                # ---- matmul2: y[ts, dc] = hT[:, ts].T @ w2[:, dc] ----
                for ts in range(TT // 128):
                    for dc in range(ND):
                        ps2 = ps2pool.tile([128, 512], F32, name="ps2", tag="ps2")
                        for f in range(KF):
                            nc.tensor.matmul(
                                ps2[:],
                                h_sb[f][:, ts * 128:(ts + 1) * 128],
                                w2_all[:, f // 8, f % 8,
                                       dc * 512:(dc + 1) * 512],
                                start=(f == 0),
                                stop=(f == KF - 1),
                            )
                        ysb = ypool.tile([128, 512], F32, name="ysb", tag="ysb")
                        last_chain = (ti == len(tiles) - 1
                                      and ts == TT // 128 - 1 and dc == ND - 1)
                        if last_chain:
                            # tail: overlap the final store with its copies,
                            # on the (idle) sync queue
                            for c0 in range(0, 512, 256):
                                nc.vector.tensor_copy(ysb[:, c0:c0 + 128],
                                                      ps2[:, c0:c0 + 128])
                                nc.vector.tensor_copy(ysb[:, c0 + 128:c0 + 256],
                                                      ps2[:, c0 + 128:c0 + 256])
                                nc.sync.dma_start(
                                    out=y_d[t0 + ts * 128:t0 + (ts + 1) * 128,
                                            dc * 512 + c0:dc * 512 + c0 + 256],
                                    in_=ysb[:, c0:c0 + 256],
                                )
                        else:
                            for c0 in range(0, 512, 128):
                                nc.vector.tensor_copy(ysb[:, c0:c0 + 128],
                                                      ps2[:, c0:c0 + 128])
                            nc.gpsimd.dma_start(
                                out=y_d[t0 + ts * 128:t0 + (ts + 1) * 128,
                                        dc * 512:(dc + 1) * 512],
                                in_=ysb[:],
                            )

    nc.compile()
    return nc


def kernel(hidden_states, selected_experts, routing_weights, w1, w2):
    global LAST_RESULTS

    hs = np.asarray(hidden_states, dtype=np.float32)
    sel = np.asarray(selected_experts)
    rw = np.asarray(routing_weights, dtype=np.float32)
    w1 = np.asarray(w1, dtype=np.float32)
    w2 = np.asarray(w2, dtype=np.float32)

    n_tokens, d_model = hs.shape
    top_k = sel.shape[1]
    n_experts, _, d_ff = w1.shape
    assert n_experts == N_CORES, "one expert per core"

    # ---- host dispatch: sort assignments by expert ----
    flat_e = np.ascontiguousarray(sel).reshape(-1).astype(np.int64)
    order = np.argsort(flat_e, kind="stable")          # assignment ids sorted by expert
    counts = np.bincount(flat_e, minlength=n_experts)
    starts = np.zeros(n_experts + 1, dtype=np.int64)
    np.cumsum(counts, out=starts[1:])
    token_of = order // top_k                          # token index per sorted assignment

    C = max(128 * int(np.ceil(counts.max() / 128)), 512)

    # per-core inputs, block-contiguous layouts (see _build_graph)
    KD = d_model // 128
    KF = d_ff // 128
    G1 = KF // 4
    G2 = KF // 8
    NT = len(_token_tiles(C))
    C512 = NT * 512 if C % 512 else C
    w1_bf = w1.astype(ml_dtypes.bfloat16)
    w2_bf = w2.astype(ml_dtypes.bfloat16)
    in_maps = []
    for e in range(n_experts):
        toks = token_of[starts[e]:starts[e + 1]]
        xpad = np.zeros((C512, d_model), dtype=ml_dtypes.bfloat16)
        if len(toks):
            xpad[:len(toks)] = hs[toks].astype(ml_dtypes.bfloat16)
        # [NT,512,KD,128] -> [NT,128,KD,512]
        xTi = np.ascontiguousarray(
            xpad.reshape(NT, 512, KD, 128).transpose(0, 3, 2, 1))
        # w1 [d_model, d_ff] -> [G1,128,KD,512]: w1i[g,p,k,c] = w1[k*128+p, g*512+c]
        w1i = np.ascontiguousarray(
            w1_bf[e].reshape(KD, 128, G1, 512).transpose(2, 1, 0, 3))
        # w2 [d_ff, d_model] -> [G2,128,8,d_model]: w2i[q,p,j,c] = w2[(8q+j)*128+p, c]
        w2i = np.ascontiguousarray(
            w2_bf[e].reshape(G2, 8, 128, d_model).transpose(0, 2, 1, 3))
        in_maps.append({"xT": xTi, "w1": w1i, "w2": w2i})

    key = (C, d_model, d_ff)
    nc = _GRAPH_CACHE.get(key)
    if nc is None:
        nc = _build_graph(C, d_model, d_ff)
        _GRAPH_CACHE[key] = nc

    res = run_bass_kernel_spmd(nc, in_maps, core_ids=list(range(N_CORES)))
    LAST_RESULTS = res

    # ---- host combine ----
    # res_sorted[p] = expert-FFN output row for sorted assignment p
    res_sorted = np.empty((n_tokens * top_k, d_model), dtype=np.float32)
    for e in range(n_experts):
        cnt = int(counts[e])
        if cnt:
            res_sorted[starts[e]:starts[e + 1]] = res.results[e]["y"][:cnt]

    inv = np.empty_like(order)
    inv[order] = np.arange(len(order))
    per_assign = res_sorted[inv].reshape(n_tokens, top_k, d_model)
    out = np.einsum("tkd,tk->td", per_assign, rw).astype(np.float32)
    return out
